# revision 32
# baseline (speedup 1.0000x reference)
"""DropPart masking kernel for Trainium2 (8 NeuronCores, data-parallel over batch).

Problem: x (64, 256, 96, 32) f32. Per sample n and channel-group g (8 groups x
32 channels), a keypoint defines a keep-box; if roll[n,g] < 0.5 the group's
channels are zeroed outside the box (box <= 16x16 in the 96x32 image), else the
group passes through unchanged.

The op multiplies ~half the (n, g) "slots" by a 0/1 mask and leaves the rest
alone.  This kernel runs IN-PLACE (the full x tensor is donated as the buffer
backing the NEFF's output), so identity slots cost ZERO HBM traffic.  All
masked-slot I/O goes through batched gpsimd indirect DMAs (gather/scatter with
an SBUF index tile, one DRAM row per index), so a core's whole data-dependent
work list is a handful of instructions with no per-slot issue overhead and no
values_load plumbing; inactive index positions carry an out-of-bounds PAD
value that the DGE bounds check silently skips, so padding costs nothing.

Host-side, samples are permuted so the per-core masked-slot counts are
balanced (the straggler core sets the kernel time), and the permutation is
inverted on the way out.

Mode "W" (default) exploits the mask structure: the keep-box spans at most 16
of the 96 image rows, i.e. at most 2 of the 6 512-element chunks per channel
row.  Per (slot, channel): gather only the <=2 box-intersecting chunks
([128, 2]-index gather, PAD for single-chunk boxes), multiply by the matching
window mask, scatter back; the remaining 4-5 chunks are pure zeros in the
output and are written by a zero-chunk scatter from a static zero tile
([128, 5] indices).  Window rows and zero rows are disjoint by construction,
so the three DMAs per item have no ordering hazards; ~480KB moves per masked
slot instead of the 768KB of a full slab read-modify-write.

Mode "B" (fallback): full-slab RMW -- gather 4 slots' 32 channel rows
([128, 3072] f32, 1.5MB), tensor_mul with a resident mask, scatter back.

dep_tracking_offset pins each indirect DMA to a disjoint fake region so the
Tile scheduler pipelines instructions instead of serializing on the whole
output tensor; items touch provably disjoint slots, so this is sound, and
same-region instructions across For_i iterations stay ordered.  Masking is
idempotent (mask in {0,1}), so the For_i(nreps) timing wrapper can repeat the
body in-place.
"""

import os

import numpy as np

import bass_rust
import concourse.bass as bass
import concourse.bacc as bacc
import concourse.tile as tile
from concourse import mybir

N, C, H, W = 64, 256, 96, 32
GROUPS = 8
P_DROP = 0.5
HW = H * W              # 3072 elements per channel image
CHS = C // GROUPS       # 32 channels per group
N_CORES = 8
NPC = N // N_CORES      # samples per core = 8
SLOTS = NPC * GROUPS    # (sample, group) slots per core = 64
ROWS = SLOTS * CHS      # 2048 channel rows of 3072 f32 per core
PAD_IDX = ROWS          # out-of-bounds row index -> DGE skips the transfer
NCHUNK = 6              # 512-element chunks per channel image (16 rows each)
ROWS6 = ROWS * NCHUNK   # 12288 chunk rows of 512 f32 per core
PAD6 = ROWS6            # out-of-bounds chunk-row index

MODE = os.environ.get("DROPPART_MODE", "W")

_F32 = mybir.dt.float32
_I32 = mybir.dt.int32


def _host_masks(key_pts: np.ndarray, roll: np.ndarray) -> np.ndarray:
    """Per-(n,g) masks [N, GROUPS, H*W] in {0,1} f32, math exactly as reference."""
    s = int(0.25 * W)
    kx = (key_pts[:, :GROUPS, 0] * np.float32(W)).astype(np.float32)
    ky = (key_pts[:, :GROUPS, 1] * np.float32(H)).astype(np.float32)
    cond = (roll[:, :GROUPS] < np.float32(P_DROP)) & (kx >= 0) & (ky >= 0)

    bx = np.floor(np.maximum(kx - s, np.float32(0.0)))
    ex = np.floor(np.minimum(kx + s, np.float32(W)))
    by = np.floor(np.maximum(ky - s, np.float32(0.0)))
    ey = np.floor(np.minimum(ky + s, np.float32(H)))

    xs = np.arange(W, dtype=np.float32)
    ys = np.arange(H, dtype=np.float32)
    inx = (xs[None, None, :] >= bx[:, :, None]) & (xs[None, None, :] < ex[:, :, None])
    iny = (ys[None, None, :] >= by[:, :, None]) & (ys[None, None, :] < ey[:, :, None])
    box = iny[:, :, :, None] & inx[:, :, None, :]  # [N, G, H, W] bool

    mask = np.where(cond[:, :, None, None], box, True)
    return mask.reshape(N, GROUPS, HW).astype(np.float32)


def _balance_perm(counts: np.ndarray) -> np.ndarray:
    """LPT-pack the 64 samples into 8 bins of exactly 8 samples each,
    balancing the per-bin masked-group totals. Returns perm: position i in
    the packed order holds original sample perm[i]; bin c = perm[8c:8c+8]."""
    order = np.argsort(-counts, kind="stable")
    bins = [[] for _ in range(N_CORES)]
    sums = np.zeros(N_CORES)
    for s in order:
        open_bins = [b for b in range(N_CORES) if len(bins[b]) < NPC]
        b = min(open_bins, key=lambda bb: (sums[bb], len(bins[bb])))
        bins[b].append(int(s))
        sums[b] += counts[s]
    return np.array([s for b in bins for s in b], dtype=np.int64)


def build_schedule(key_pts: np.ndarray, roll: np.ndarray):
    """Host schedule for the indirect DMAs.

    Returns (w_items, ins: dict name -> per-core list of host arrays, perm).
    Item k's partition p covers slot active[4k + p//32], channel p%32.
    """
    masks = _host_masks(key_pts, roll)  # [N, G, HW] f32 0/1
    masked = masks.min(axis=2) < 1.0  # [N, G] bool
    perm = _balance_perm(masked.sum(axis=1).astype(np.float64))
    m_core = masks[perm].reshape(N_CORES, SLOTS, HW)
    active = [[sl for sl in range(SLOTS) if m_core[c, sl].min() < 1.0]
              for c in range(N_CORES)]
    w_items = max(1, max(-(-len(a) // 4) for a in active))

    ch = np.arange(CHS, dtype=np.int32)
    if MODE == "B":
        idxs, mpks = [], []
        for c in range(N_CORES):
            idx = np.full((128, w_items), PAD_IDX, dtype=np.int32)
            mpk = np.zeros((128, w_items * HW), dtype=np.float32)
            for j, sl in enumerate(active[c]):
                k, q = divmod(j, 4)
                rows = slice(CHS * q, CHS * (q + 1))
                idx[rows, k] = sl * CHS + ch
                mpk[rows, k * HW : (k + 1) * HW] = m_core[c, sl][None, :]
            idxs.append(idx)
            mpks.append(mpk)
        return w_items, {"idx": idxs, "mpk": mpks}, perm

    # MODE "W": per (slot, channel), a CONSECUTIVE pair of window chunks
    # [c0, c0+1] covers the <=16-row box (the HW DGE auto-increments
    # multi-index transfers from idx[p,0], so indices within an instruction
    # must be consecutive -- host writes the actual consecutive values so the
    # functional interpreter agrees).  The other 4 chunks are pure zeros:
    # a prefix run [0, c0) and a suffix run [c0+2, 6).  Run lengths must be
    # uniform within an instruction, so slots are grouped into items by c0
    # ("class"); a zero run is a [128, 1]-index scatter whose source size
    # (run_len * 512) sets the auto-increment count.  Zero rows never touch
    # window rows, so the only ordering is gather -> scatter per item.
    grp = [[[] for _ in range(NCHUNK - 1)] for _ in range(N_CORES)]
    c0s = {}
    for c in range(N_CORES):
        for sl in active[c]:
            m6 = m_core[c, sl].reshape(NCHUNK, 512)
            nzc = np.nonzero(m6.any(axis=1))[0]
            assert 1 <= len(nzc) <= 2 and nzc[-1] - nzc[0] == len(nzc) - 1, nzc
            c0 = min(int(nzc[0]), NCHUNK - 2)
            grp[c][c0].append(sl)
            c0s[(c, sl)] = c0
    w_key = tuple(max(-(-len(grp[c][q]) // 4) for c in range(N_CORES))
                  for q in range(NCHUNK - 1))
    T = sum(w_key)

    idxws, idxzps, idxzss, wmpks = [], [], [], []
    for c in range(N_CORES):
        idxw = np.full((128, 2 * T), PAD6, dtype=np.int32)
        idxzp = np.full((128, T), PAD6, dtype=np.int32)
        idxzs = np.full((128, T), PAD6, dtype=np.int32)
        wmpk = np.zeros((128, T * 1024), dtype=np.float32)
        r0 = 0
        for q in range(NCHUNK - 1):
            for j, sl in enumerate(grp[c][q]):
                r = r0 + j // 4
                rows = slice(CHS * (j % 4), CHS * (j % 4 + 1))
                m6 = m_core[c, sl].reshape(NCHUNK, 512)
                base = (sl * CHS + ch) * NCHUNK
                for b in range(2):
                    idxw[rows, 2 * r + b] = base + q + b
                    wmpk[rows, r * 1024 + 512 * b : r * 1024 + 512 * (b + 1)] = \
                        m6[q + b][None, :]
                if q > 0:
                    idxzp[rows, r] = base  # prefix run [0, q)
                if q < NCHUNK - 2:
                    idxzs[rows, r] = base + q + 2  # suffix run [q+2, 6)
            r0 += w_key[q]
        idxws.append(idxw)
        idxzps.append(idxzp)
        idxzss.append(idxzs)
        wmpks.append(wmpk)
    return w_key, {"idxw": idxws, "idxzp": idxzps, "idxzs": idxzss,
                   "wmpk": wmpks}, perm


def _build_module(w_items: int, mode: str = MODE):
    """One SPMD module; all data movement via 128-index indirect DMAs."""
    nc = bacc.Bacc("TRN2", target_bir_lowering=False, debug=False)

    reps_d = nc.dram_tensor("nreps", [1, 1], _I32, kind="ExternalInput").ap()
    if mode == "B":
        o_d = nc.dram_tensor("out", [ROWS, HW], _F32, kind="ExternalOutput").ap()
        idx_d = nc.dram_tensor("idx", [128, w_items], _I32, kind="ExternalInput").ap()
        mpk_d = nc.dram_tensor("mpk", [128, w_items * HW], _F32, kind="ExternalInput").ap()
        nrow, pad = ROWS, PAD_IDX
    else:
        w_key = w_items  # tuple of per-c0-class item capacities
        T = sum(w_key)
        items = [(q, k) for q in range(NCHUNK - 1) for k in range(w_key[q])]
        o_d = nc.dram_tensor("out", [ROWS6, 512], _F32, kind="ExternalOutput").ap()
        idxw_d = nc.dram_tensor("idxw", [128, 2 * T], _I32, kind="ExternalInput").ap()
        idxzp_d = nc.dram_tensor("idxzp", [128, T], _I32, kind="ExternalInput").ap()
        idxzs_d = nc.dram_tensor("idxzs", [128, T], _I32, kind="ExternalInput").ap()
        wmpk_d = nc.dram_tensor("wmpk", [128, T * 1024], _F32, kind="ExternalInput").ap()
        nrow, pad = ROWS6, PAD6

    def o_fake(r):
        # Full-tensor AP (offset must be 0 for the indirect lowering), but a
        # disjoint fake dep region per item: items touch provably disjoint
        # rows, so dropping the scheduler's whole-tensor serialization is
        # sound; same-region instructions across For_i iterations stay
        # ordered.
        a = o_d[:].copy()
        a.dep_tracking_offset = (r + 1) * nrow * (HW if mode == "B" else 512)
        return a



    with tile.TileContext(nc) as tc:
        with (
            tc.tile_pool(name="consts", bufs=1) as consts,
            tc.tile_pool(name="xpool", bufs=(4 if mode == "B" else 1)) as xpool,
        ):
            rtile = consts.tile([1, 1], _I32)
            nc.sync.dma_start(rtile[:], reps_d[:])
            if mode == "B":
                it = consts.tile([128, w_items], _I32)
                nc.sync.dma_start(it[:], idx_d[:])
                mt = consts.tile([128, w_items * HW], _F32)
                nc.sync.dma_start(mt[:], mpk_d[:])
            else:
                # The DGE misreads sliced offset APs, so each indirect DMA
                # gets its own full [128, B] index tile.
                itws, itzps, itzss = [], [], []
                for r, (q, k) in enumerate(items):
                    itw = consts.tile([128, 2], _I32, name=f"itw{r}")
                    nc.sync.dma_start(itw[:], idxw_d[:, 2 * r : 2 * (r + 1)])
                    itws.append(itw)
                    itzp = itzs_ = None
                    if q > 0:
                        itzp = consts.tile([128, 1], _I32, name=f"itzp{r}")
                        nc.scalar.dma_start(itzp[:], idxzp_d[:, r : r + 1])
                    if q < NCHUNK - 2:
                        itzs_ = consts.tile([128, 1], _I32, name=f"itzs{r}")
                        nc.scalar.dma_start(itzs_[:], idxzs_d[:, r : r + 1])
                    itzps.append(itzp)
                    itzss.append(itzs_)
                wmt = consts.tile([128, T * 1024], _F32)
                nc.sync.dma_start(wmt[:], wmpk_d[:])
                zt = consts.tile([128, (NCHUNK - 2) * 512], _F32)
                nc.vector.memset(zt[:], 0.0)

            with tc.For_i(0, nc.values_load(rtile[0:1, 0:1]), 1):
                if mode == "B":
                    for k in range(w_items):
                        ioff = bass.IndirectOffsetOnAxis(ap=it[:, k : k + 1], axis=0)
                        xt = xpool.tile([128, HW], _F32)
                        nc.gpsimd.indirect_dma_start(
                            out=xt[:], out_offset=None,
                            in_=o_fake(k), in_offset=ioff,
                            bounds_check=nrow - 1, oob_is_err=False)
                        nc.vector.tensor_mul(xt[:], xt[:], mt[:, k * HW : (k + 1) * HW])
                        nc.gpsimd.indirect_dma_start(
                            out=o_fake(k), out_offset=ioff,
                            in_=xt[:], in_offset=None,
                            bounds_check=nrow - 1, oob_is_err=False)
                else:
                    # Three phases: all window gathers, then all zero runs
                    # (+ muls on DVE), then all window scatters.  Item r's
                    # DMAs share dep region r (serialized in this order);
                    # phase-majoring keeps the in-order Pool queue from
                    # stalling on a region wait while later items'
                    # instructions could already be generating/transferring.
                    # Zero runs land on chunks outside the window pair, so
                    # nothing is written twice.
                    wts = []
                    for r, (q, k) in enumerate(items):
                        woff = bass.IndirectOffsetOnAxis(ap=itws[r][:], axis=0)
                        wt = xpool.tile([128, 1024], _F32, name=f"wt{r}")
                        nc.gpsimd.indirect_dma_start(
                            out=wt[:], out_offset=None,
                            in_=o_fake(r), in_offset=woff,
                            bounds_check=nrow - 1, oob_is_err=False)
                        wts.append(wt)
                    for r, (q, k) in enumerate(items):
                        if q > 0:  # prefix zeros: q chunks from row base
                            nc.gpsimd.indirect_dma_start(
                                out=o_fake(r),
                                out_offset=bass.IndirectOffsetOnAxis(
                                    ap=itzps[r][:], axis=0),
                                in_=zt[:, : q * 512], in_offset=None,
                                bounds_check=nrow - 1, oob_is_err=False)
                        if q < NCHUNK - 2:  # suffix zeros: 4-q chunks
                            nc.gpsimd.indirect_dma_start(
                                out=o_fake(r),
                                out_offset=bass.IndirectOffsetOnAxis(
                                    ap=itzss[r][:], axis=0),
                                in_=zt[:, : (NCHUNK - 2 - q) * 512],
                                in_offset=None,
                                bounds_check=nrow - 1, oob_is_err=False)
                        nc.vector.tensor_mul(wts[r][:], wts[r][:],
                                             wmt[:, r * 1024 : (r + 1) * 1024])
                    for r, (q, k) in enumerate(items):
                        woff = bass.IndirectOffsetOnAxis(ap=itws[r][:], axis=0)
                        nc.gpsimd.indirect_dma_start(
                            out=o_fake(r), out_offset=woff,
                            in_=wts[r][:], in_offset=None,
                            bounds_check=nrow - 1, oob_is_err=False)

    nc.compile()
    return nc


_MODULES: dict = {}


def _get_module(w_items: int):
    key = (w_items, MODE)
    if key not in _MODULES:
        _MODULES[key] = _build_module(w_items)
    return _MODULES[key]


def make_runner(nc):
    """jit'd shard_map runner over 8 cores with the 'out' buffer donated.

    Returns (fn, mesh, order): fn(*ins_in_order, out_g) -> (out_g,); out_g is
    consumed (donated); chain calls by passing the previous result.
    """
    import jax
    from jax.sharding import Mesh, PartitionSpec
    from jax.experimental.shard_map import shard_map
    from concourse.bass2jax import (
        _bass_exec_p,
        install_neuronx_cc_hook,
        partition_id_tensor,
    )

    install_neuronx_cc_hook()
    partition_name = nc.partition_id_tensor.name if nc.partition_id_tensor else None

    in_names, out_names, out_avals = [], [], []
    for alloc in nc.m.functions[0].allocations:
        if not isinstance(alloc, mybir.MemoryLocationSet):
            continue
        name = alloc.memorylocations[0].name
        if alloc.kind == "ExternalInput":
            if name != partition_name:
                in_names.append(name)
        elif alloc.kind == "ExternalOutput":
            out_names.append(name)
            out_avals.append(jax.core.ShapedArray(tuple(alloc.tensor_shape),
                                                  mybir.dt.np(alloc.dtype)))
    assert out_names == ["out"]
    n_in = len(in_names)
    all_names = tuple(in_names) + ("out",)
    if partition_name is not None:
        all_names = all_names + (partition_name,)

    def _body(*args):
        operands = list(args[:n_in + 1])
        if partition_name is not None:
            operands.append(partition_id_tensor())
        (res,) = _bass_exec_p.bind(
            *operands,
            out_avals=tuple(out_avals),
            in_names=all_names,
            out_names=("out",),
            lowering_input_output_aliases=(),
            sim_require_finite=False, sim_require_nnan=False, nc=nc)
        return (res,)

    mesh = Mesh(np.asarray(jax.devices()[:N_CORES]), ("core",))
    specs = (PartitionSpec("core"),) * (n_in + 1)
    fn = jax.jit(
        shard_map(_body, mesh=mesh, in_specs=specs,
                  out_specs=(PartitionSpec("core"),), check_rep=False),
        donate_argnums=(n_in,), keep_unused=True)
    return fn, mesh, list(in_names)


def device_inputs(ins: dict, mesh, nreps: int = 1):
    """device_put the per-core host arrays (plus the nreps scalar)."""
    import jax
    from jax.sharding import NamedSharding, PartitionSpec

    sharding = NamedSharding(mesh, PartitionSpec("core"))
    d = {name: jax.device_put(np.concatenate(arrs, axis=0), sharding)
         for name, arrs in ins.items()}
    d["nreps"] = jax.device_put(np.full((N_CORES, 1), nreps, np.int32), sharding)
    return d, sharding


def kernel(x: np.ndarray, key_pts: np.ndarray, roll: np.ndarray, **_kw) -> np.ndarray:
    import jax

    x = np.ascontiguousarray(np.asarray(x, dtype=np.float32))
    key_pts = np.asarray(key_pts, dtype=np.float32)
    roll = np.asarray(roll, dtype=np.float32)

    w_items, ins, perm = build_schedule(key_pts, roll)
    nc = _get_module(w_items)
    fn, mesh, order = make_runner(nc)
    d, sharding = device_inputs(ins, mesh)
    out_g = jax.device_put(x[perm].reshape(nc_out_shape()), sharding)

    res = fn(*[d[n] for n in order], out_g)[0]
    res = np.asarray(res).reshape(N, C, H, W)
    final = np.empty_like(res)
    final[perm] = res
    return final


def nc_out_shape():
    return (N_CORES * (ROWS if MODE == "B" else ROWS6), HW if MODE == "B" else 512)
